# revision 1
# baseline (speedup 1.0000x reference)
"""MoE grouped-GEMM kernel for Trainium2 (8 NeuronCores, expert-parallel).

Problem: x [16384, 1024] fp16, expert_indices [16384] int32 (0..7),
weights [8, 1024, 4096] fp16. Output: fp16 [16384, 4096] in sorted-token
order (stable sort by expert), fp32 accumulation.

Sharding: the host performs the argsort/bincount dispatch (that IS the
sharding step) and gives core e the tokens routed to expert e as a
pre-transposed xT [K, Mpad] fp16 block plus that expert's weights
[K, N]. Every core runs the identical dense-GEMM program (token counts
padded to a common multiple of 128), so a single SPMD NEFF drives all 8
cores with no device-side collectives. The host concatenates the
per-expert output blocks, which is exactly sorted-token order.
"""

import numpy as np

_NCORES = 8


def _build_program(T, K, N):
    """Dense GEMM per core: out[Mpad, N] = xT.T @ w, fp32 PSUM accumulation.

    Layout per core:
      xT [K, Mpad] fp16  (x pre-transposed on host so K lands on partitions)
      w  [K, N]   fp16
      out [Mpad, N] fp16, Mpad = T*128

    PE mapping: stationary lhsT = xT k-tile [128, 128], moving rhs = w
    [128, 512] slice, PSUM [128m, 512n] fp32 accumulated over K/128
    k-tiles. PSUM is split into two 4-bank halves (bufs=2) so the
    DVE fp32->fp16 eviction of one half overlaps matmuls in the other.
    """
    from concourse import bacc, bass, tile
    import concourse.mybir as mybir
    from concourse.vector_clock import ScopedClock

    class _FastExitTC(tile.TileContext):
        # The stock exit path is drain -> barrier -> sem clears ->
        # barrier (~5us). The clears and second barrier only matter if
        # the NEFF is re-executed with warm semaphore state; this kernel
        # compiles a fresh NEFF per call and executes it once, so end
        # after the first barrier.
        def _drain_and_barrier(self, tick_clock, wait_clock):
            drain_inst = self.nc.sync.drain()
            wait_clock.add_sem_waits(
                drain_inst.ins, ScopedClock({None: tick_clock.global_clock})
            )
            self.nc.all_engine_barrier()
            popped = self.nc._tile_sem_poison_stack.pop()
            assert popped is self._sem_poison

    f16 = mybir.dt.float16
    f32 = mybir.dt.float32
    Mpad = T * 128
    KT = K // 128            # k-tiles (contraction)
    NB = 512                 # one PSUM bank of fp32
    NH = 2048                # psum half (4 banks)
    nhalves = N // NH

    # Skip the ctor-time all-engine barrier (~3.4us of engine-arrival
    # stagger plus serialization before the first DMA can issue). All
    # cross-engine ordering in this kernel goes through semaphores, which
    # the runtime zeroes at NEFF load, and the NEFF runs exactly once per
    # compile — the barrier only guards warm-state reuse. The patch is
    # restored before TileContext exit, which still emits its barrier.
    _orig_aeb = bass.Bass.all_engine_barrier
    bass.Bass.all_engine_barrier = lambda self, *a, **k: None
    try:
        nc = bacc.Bacc(
            "TRN2",
            target_bir_lowering=False,
            debug=False,
            num_devices=_NCORES,
            # pure data-parallel SPMD: no instruction reads the core id
            enable_partition_id=False,
        )
    finally:
        bass.Bass.all_engine_barrier = _orig_aeb
    xT = nc.dram_tensor("xT", [K, Mpad], f16, kind="ExternalInput").ap()
    w = nc.dram_tensor("w", [K, N], f16, kind="ExternalInput").ap()
    out = nc.dram_tensor("out", [Mpad, N], f16, kind="ExternalOutput").ap()

    with _FastExitTC(nc) as tc:
        with (
            tc.tile_pool(name="xw", bufs=1) as xw,
            tc.tile_pool(name="op", bufs=6) as op,
            tc.tile_pool(name="pp", bufs=2, space=bass.MemorySpace.PSUM) as pp,
        ):
            # Whole x and w stay SBUF-resident (~99KB/partition total).
            # PE clock-gate warm-up: 10 matmuls on memset tiles issued
            # during the initial DMA wait so the HAM un-throttles (1.2 ->
            # 2.4GHz takes ~3.4us of sustained PE activity) before the
            # first real matmul. The dummy psum slot is recycled by the
            # pool before any real accumulation starts.
            # gpsimd exits the entry butterfly earliest, so its memsets
            # unblock the dummy burst ~1.5us sooner than Vector's would;
            # 7 dummies span ~4us of PE activity, enough to flip the HAM
            # clock gate before the first real matmul issues (~9.6us)
            zs = xw.tile([128, 128], f16, tag="zstat")
            zm = xw.tile([128, NB], f16, tag="zmov")
            nc.gpsimd.memset(zs[:], 0.0)
            nc.gpsimd.memset(zm[:], 0.0)
            pwarm = pp.tile([128, NH], f32, tag="ps")
            for i in range(7):
                nc.tensor.matmul(
                    pwarm[:, 0:NB], zs[:], zm[:], start=(i == 0), stop=(i == 6)
                )

            # x strips are split three ways by first-use time: head
            # (t=0..1, so the PE has two tiles of work per arriving w
            # strip from the very start), early rest (t=2..5), late rest
            # (t>=6). Everything rides the sync HWDGE rail (it alone
            # reaches ~400GB/s; a second concurrent rail just splits HBM
            # bandwidth), issued in exact first-use order. w(k=0,h=0) is
            # further chunked into 512-col pieces so the first matmul's
            # deps are only ~96KB deep. h=1 w strips aren't needed until
            # ~125us.
            TH = min(2, T)  # tiles covered by the head strips
            TE = min(6, T)  # tiles covered by head + early rest
            xheads = []
            xearly = []
            xlate = []
            ws = [[None] * nhalves for _ in range(KT)]
            w00c = []
            xh = xw.tile([128, TH * 128], f16, tag="xh0")
            nc.sync.dma_start(xh[:], xT[0:128, 0 : TH * 128])
            xheads.append(xh)
            for n in range(NH // NB):
                c = xw.tile([128, NB], f16, tag=f"w00c{n}")
                nc.sync.dma_start(c[:], w[0:128, n * NB : (n + 1) * NB])
                w00c.append(c)
            for k in range(1, KT):
                xh = xw.tile([128, TH * 128], f16, tag=f"xh{k}")
                nc.sync.dma_start(xh[:], xT[k * 128 : (k + 1) * 128, 0 : TH * 128])
                xheads.append(xh)
                wt = xw.tile([128, NH], f16, tag=f"w{k}h0")
                nc.sync.dma_start(wt[:], w[k * 128 : (k + 1) * 128, 0:NH])
                ws[k][0] = wt
            if TE > TH:
                for k in range(KT):
                    xe = xw.tile([128, (TE - TH) * 128], f16, tag=f"xe{k}")
                    nc.sync.dma_start(
                        xe[:], xT[k * 128 : (k + 1) * 128, TH * 128 : TE * 128]
                    )
                    xearly.append(xe)
            if T > TE:
                for k in range(KT):
                    xl = xw.tile([128, (T - TE) * 128], f16, tag=f"xl{k}")
                    nc.sync.dma_start(
                        xl[:], xT[k * 128 : (k + 1) * 128, TE * 128 : Mpad]
                    )
                    xlate.append(xl)
            for h in range(1, nhalves):
                for k in range(KT):
                    wt = xw.tile([128, NH], f16, tag=f"w{k}h{h}")
                    nc.sync.dma_start(
                        wt[:], w[k * 128 : (k + 1) * 128, h * NH : (h + 1) * NH]
                    )
                    ws[k][h] = wt

            def lhs_for(k, t):
                if t < TH:
                    return xheads[k][:, t * 128 : (t + 1) * 128]
                if t < TE:
                    return xearly[k][:, (t - TH) * 128 : (t - TH + 1) * 128]
                return xlate[k][:, (t - TE) * 128 : (t - TE + 1) * 128]

            def rhs_for(k, h, n0):
                # n0 is the 512-col slice index within the h-half
                if h == 0 and k == 0:
                    return w00c[n0][:]
                return ws[k][h][:, n0 * NB : (n0 + 1) * NB]

            NQ = 1024  # output eviction chunk (cast + store pipelined)
            for h in range(nhalves):
                for t in range(T):
                    last = h == nhalves - 1 and t == T - 1
                    # the final tile accumulates in two 2-bank groups so
                    # its eviction (the kernel tail) is half as deep
                    npieces = 2 if last else 1
                    pw = NH // npieces
                    for p in range(npieces):
                        ps = pp.tile([128, pw], f32, tag="ps")
                        for k in range(KT):
                            lhs = lhs_for(k, t)
                            for n in range(pw // NB):
                                nc.tensor.matmul(
                                    ps[:, n * NB : (n + 1) * NB],
                                    lhs,
                                    rhs_for(k, h, p * (pw // NB) + n),
                                    start=(k == 0),
                                    stop=(k == KT - 1),
                                )
                        nq = 512 if last else NQ
                        for q in range(pw // nq):
                            ot = op.tile([128, NQ], f16, tag="ot")
                            nc.vector.tensor_copy(
                                ot[:, :nq], ps[:, q * nq : (q + 1) * nq]
                            )
                            # alternate output rails (sync HWDGE ran
                            # ~105GB/s, scalar ~80GB/s; either alone
                            # barely keeps up)
                            # parity chosen so the very last chunk (odd
                            # index) rides the faster sync rail — the
                            # final store gates the exit drain
                            eng = (
                                nc.sync
                                if (t * nhalves + h + p + q) % 2
                                else nc.scalar
                            )
                            col0 = h * NH + p * pw + q * nq
                            eng.dma_start(
                                out[t * 128 : (t + 1) * 128, col0 : col0 + nq],
                                ot[:, :nq],
                            )
    nc.compile()
    return nc


# test.py reads these after a call for timing/trace introspection
last_results = None


def kernel(x, expert_indices, weights):
    x = np.asarray(x)
    ei = np.asarray(expert_indices)
    w = np.asarray(weights)
    M, K = x.shape
    E, K2, N = w.shape
    assert K == K2 and E == _NCORES

    counts = np.bincount(ei, minlength=E)
    T = max(1, -(-int(counts.max()) // 128))
    Mpad = T * 128
    order = np.argsort(ei, kind="stable")
    x_sorted = x[order]
    offs = np.zeros(E + 1, dtype=np.int64)
    np.cumsum(counts, out=offs[1:])

    in_maps = []
    for e in range(E):
        blk = x_sorted[offs[e] : offs[e + 1]]
        xeT = np.zeros((K, Mpad), dtype=np.float16)
        xeT[:, : blk.shape[0]] = blk.T
        in_maps.append({"xT": xeT, "w": np.ascontiguousarray(w[e])})

    nc = _build_program(T, K, N)

    from concourse.bass_utils import run_bass_kernel_spmd

    res = run_bass_kernel_spmd(nc, in_maps, list(range(E)))
    global last_results
    last_results = res

    out = np.empty((M, N), dtype=np.float16)
    for e in range(E):
        out[offs[e] : offs[e + 1]] = res.results[e]["out"][: counts[e]]
    return out

